# revision 17
# baseline (speedup 1.0000x reference)
"""CTC greedy decoder (K.ctc_decode greedy path) on 8 Trainium2 NeuronCores.

Strategy (pure data parallel, batch sharded 8 ways):
  Device (per core, input [4096, 11000] f32 = 8 batch rows x 512 timesteps):
    for each tile of 128 rows:
      - segmented reduce_max over 88 chunks of 125 -> chunk maxes [128, 88]
      - Max8 over chunk maxes -> global row max (value)
      - MaxIndex over chunk maxes -> winning chunk id (first occurrence)
      - indirect DMA gather of the winning 125-element chunk per row
      - MaxIndex within the gathered chunk -> within-chunk argmax position
  Host: combine (chunk_id * 125 + within) -> argmax token per (b, t);
    log/score computation, merge-repeats, drop-blank, left-pack are O(B*T).

argmax(p) == argmax(log(p + eps)) since log is monotonic; max log-prob
per step == log(max p + eps) for the same reason.
"""

import sys

import numpy as np

for _p in ("/opt/trn_rl_repo",):
    if _p not in sys.path:
        sys.path.insert(0, _p)

B, T, C = 64, 512, 11000
N_CORES = 8
B_PER_CORE = B // N_CORES  # 8
ROWS = B_PER_CORE * T  # 4096 rows per core
P = 128  # SBUF partitions
NTILES = ROWS // P  # 32
CHUNK = 125  # 11000 = 88 * 125
NCHUNK = C // CHUNK  # 88
EPS = np.float32(1e-7)
BLANK = C - 1

_NC = None
LAST_RESULTS = None


def _build_nc():
    import concourse.bass as bass
    import concourse.tile as tile
    from concourse import bacc, mybir

    nc = bacc.Bacc("TRN2", target_bir_lowering=False, debug=False)
    x_h = nc.dram_tensor("x", [ROWS, C], mybir.dt.float32, kind="ExternalInput")
    om_h = nc.dram_tensor(
        "out_max", [P, NTILES * 8], mybir.dt.float32, kind="ExternalOutput"
    )
    oc_h = nc.dram_tensor(
        "out_cidx", [P, NTILES * 8], mybir.dt.uint32, kind="ExternalOutput"
    )
    ow_h = nc.dram_tensor(
        "out_widx", [P, NTILES * 8], mybir.dt.uint32, kind="ExternalOutput"
    )
    x = x_h.ap()
    x_flat = x.rearrange("r (c e) -> (r c) e", e=CHUNK)

    with tile.TileContext(nc) as tc:
        with (
            tc.tile_pool(name="xin", bufs=4) as xpool,
            tc.tile_pool(name="seg", bufs=2) as segpool,
            tc.tile_pool(name="gath", bufs=3) as gpool,
            tc.tile_pool(name="ofs", bufs=2) as opool,
            tc.tile_pool(name="persist", bufs=1) as ppool,
        ):
            res_max = ppool.tile([P, NTILES * 8], mybir.dt.float32)
            res_cidx = ppool.tile([P, NTILES * 8], mybir.dt.uint32)
            res_widx = ppool.tile([P, NTILES * 8], mybir.dt.uint32)
            # last tile's widx is host-refined; zero its slice so the final
            # DMA never reads uninitialized SBUF (keeps CoreSim strict checks
            # green; hardware output buffers are pre-zeroed anyway)
            nc.vector.memset(res_widx[:, (NTILES - 1) * 8 :], 0)
            # base_tbl[p, t] = (t*128 + p) * NCHUNK : row index into x_flat
            base_tbl = ppool.tile([P, NTILES], mybir.dt.uint32)
            nc.gpsimd.iota(
                base_tbl[:],
                pattern=[[P * NCHUNK, NTILES]],
                base=0,
                channel_multiplier=NCHUNK,
            )

            # (max8_ap, gath_tile, widx8_ap) of the previous iteration: the
            # within-chunk MaxIndex is software-pipelined one iteration late so
            # the DVE never stalls on the indirect-gather round trip.
            pending = None
            for t in range(NTILES):
                segmax = segpool.tile([P, NCHUNK], mybir.dt.float32)
                if t < NTILES - 2:
                    # split each tile into PARTITION halves, one per HWDGE
                    # ring (SP / ACT): the rings stream disjoint DRAM regions
                    # (rows p<64 / p>=64) into complementary SBUF ports, so
                    # tiles complete sequentially (no end-of-stream DVE
                    # backlog) without two rings thrashing the same DRAM rows
                    xt = xpool.tile([P, C], mybir.dt.float32, tag="xt")
                    nc.sync.dma_start(
                        xt[0 : P // 2, :], x[t * P : t * P + P // 2, :]
                    )
                    nc.scalar.dma_start(
                        xt[P // 2 : P, :], x[t * P + P // 2 : (t + 1) * P, :]
                    )
                    nc.vector.reduce_max(
                        out=segmax[:],
                        in_=xt[:].rearrange("p (c e) -> p c e", e=CHUNK),
                        axis=mybir.AxisListType.X,
                    )
                else:
                    # last two tiles: quarter the loads into SEPARATE tiles
                    # (shared slot tag -> no extra SBUF, per-quarter deps) so
                    # the reduces overlap the remaining stream and the DVE
                    # enters the tail with no backlog
                    QS = 4
                    cq = NCHUNK // QS  # 22 chunks per quarter
                    wq = cq * CHUNK  # 2750 columns
                    for q in range(QS):
                        xq = xpool.tile([P, wq], mybir.dt.float32, tag="xt")
                        dma_eng = nc.sync if q % 2 == 0 else nc.scalar
                        dma_eng.dma_start(
                            xq[:],
                            x[t * P : (t + 1) * P, q * wq : (q + 1) * wq],
                        )
                        nc.vector.reduce_max(
                            out=segmax[:, q * cq : (q + 1) * cq],
                            in_=xq[:].rearrange("p (c e) -> p c e", e=CHUNK),
                            axis=mybir.AxisListType.X,
                        )

                max8 = res_max[:, t * 8 : (t + 1) * 8]
                nc.vector.max(out=max8, in_=segmax[:])
                cidx8 = res_cidx[:, t * 8 : (t + 1) * 8]
                nc.vector.max_index(cidx8, max8, segmax[:])

                if t < NTILES - 1:
                    # last tile's within-chunk argmax is refined on the host
                    # (128 rows/core) so no gather round trip lands in the tail
                    gofs = opool.tile([P, 1], mybir.dt.uint32)
                    nc.vector.tensor_tensor(
                        out=gofs[:],
                        in0=base_tbl[:, t : t + 1],
                        in1=cidx8[:, :1],
                        op=mybir.AluOpType.add,
                    )

                    gath = gpool.tile([P, CHUNK], mybir.dt.float32)
                    nc.gpsimd.indirect_dma_start(
                        out=gath[:],
                        out_offset=None,
                        in_=x_flat,
                        in_offset=bass.IndirectOffsetOnAxis(ap=gofs[:, :1], axis=0),
                    )
                if pending is not None:
                    p_max8, p_gath, p_widx8 = pending
                    nc.vector.max_index(p_widx8, p_max8, p_gath[:])
                if t < NTILES - 1:
                    pending = (max8, gath, res_widx[:, t * 8 : (t + 1) * 8])
                else:
                    pending = None

            # res_widx complete before the stream tail -> this DMA fires early
            nc.sync.dma_start(ow_h.ap()[:, :], res_widx[:])
            nc.sync.dma_start(om_h.ap()[:, :], res_max[:])
            nc.sync.dma_start(oc_h.ap()[:, :], res_cidx[:])
    nc.compile()
    return nc


def get_nc():
    global _NC
    if _NC is None:
        _NC = _build_nc()
    return _NC


def _postprocess(tok, maxp):
    """Mirror of the reference merge/pack given per-step argmax + max prob."""
    max_lp = np.log(maxp + EPS)  # == max_c log(p + eps), f32
    scores = -np.sum(max_lp, axis=1, keepdims=True, dtype=np.float32)

    prev = np.empty_like(tok)
    prev[:, 0] = -1
    prev[:, 1:] = tok[:, :-1]
    keep = (tok != prev) & (tok != BLANK)
    pos = np.cumsum(keep, axis=1, dtype=np.int64) - 1
    idx = np.where(keep, pos, T)  # dropped tokens scatter to dump column T
    out = np.full((B, T + 1), -1, dtype=np.int32)
    out[np.arange(B)[:, None], idx] = tok
    decoded = out[:, :T]
    return decoded, scores


def core_outputs_to_maxtok(out_max, out_cidx, out_widx, shard):
    """[P, NTILES*8] device outputs + the core's input shard ->
    (maxv [ROWS], tok [ROWS]) in row order r = t*P + p."""
    mx = out_max[:, ::8]  # [P, NTILES] slot 0 of each Max8 group
    ci = out_cidx[:, ::8].astype(np.int64)
    wi = out_widx[:, ::8].astype(np.int64).copy()
    # last tile's within-chunk index is not computed on device; refine from
    # the winning chunk (P rows x CHUNK elements) on host
    tl = NTILES - 1
    rows = tl * P + np.arange(P)
    cols = ci[:, tl][:, None] * CHUNK + np.arange(CHUNK)[None, :]
    wi[:, tl] = shard[rows[:, None], cols].argmax(axis=1)
    gi = ci * CHUNK + wi  # [P, NTILES] global argmax
    # row r = t*128 + p  ->  [p, t] -> transpose -> flat r
    return mx.T.reshape(-1), gi.T.reshape(-1)


def kernel(inputs):
    global LAST_RESULTS
    from concourse.bass_utils import run_bass_kernel_spmd

    x = np.ascontiguousarray(np.asarray(inputs, dtype=np.float32)).reshape(B, T, C)
    nc = get_nc()
    shards = x.reshape(N_CORES, ROWS, C)
    in_maps = [{"x": shards[k]} for k in range(N_CORES)]
    res = run_bass_kernel_spmd(nc, in_maps, core_ids=list(range(N_CORES)))
    LAST_RESULTS = res

    maxp = np.empty((N_CORES, B_PER_CORE, T), np.float32)
    tok = np.empty((N_CORES, B_PER_CORE, T), np.int32)
    for k in range(N_CORES):
        rk = res.results[k]
        mv, gi = core_outputs_to_maxtok(
            rk["out_max"], rk["out_cidx"], rk["out_widx"], shards[k]
        )
        maxp[k] = mv.reshape(B_PER_CORE, T)
        tok[k] = gi.reshape(B_PER_CORE, T).astype(np.int32)

    return _postprocess(tok.reshape(B, T), maxp.reshape(B, T))


# revision 18
# speedup vs baseline: 1.8386x; 1.8386x over previous
"""CTC greedy decoder (K.ctc_decode greedy path) on 8 Trainium2 NeuronCores.

Strategy (pure data parallel, batch sharded 8 ways):
  Device (per core, input [4096, 11000] f32 = 8 batch rows x 512 timesteps):
    for each tile of 128 rows:
      - segmented reduce_max over 88 chunks of 125 -> chunk maxes [128, 88]
      - Max8 over chunk maxes -> global row max (value)
      - MaxIndex over chunk maxes -> winning chunk id (first occurrence)
      - indirect DMA gather of the winning 125-element chunk per row
      - MaxIndex within the gathered chunk -> within-chunk argmax position
  Host: combine (chunk_id * 125 + within) -> argmax token per (b, t);
    log/score computation, merge-repeats, drop-blank, left-pack are O(B*T).

argmax(p) == argmax(log(p + eps)) since log is monotonic; max log-prob
per step == log(max p + eps) for the same reason.
"""

import sys

import numpy as np

for _p in ("/opt/trn_rl_repo",):
    if _p not in sys.path:
        sys.path.insert(0, _p)

B, T, C = 64, 512, 11000
N_CORES = 8
B_PER_CORE = B // N_CORES  # 8
ROWS = B_PER_CORE * T  # 4096 rows per core
P = 128  # SBUF partitions
NTILES = ROWS // P  # 32
CHUNK = 125  # 11000 = 88 * 125
NCHUNK = C // CHUNK  # 88
EPS = np.float32(1e-7)
BLANK = C - 1

_NC = None
LAST_RESULTS = None


def _build_nc():
    import concourse.bass as bass
    import concourse.tile as tile
    from concourse import bacc, mybir

    nc = bacc.Bacc("TRN2", target_bir_lowering=False, debug=False)
    x_h = nc.dram_tensor("x", [ROWS, C], mybir.dt.float32, kind="ExternalInput")
    om_h = nc.dram_tensor(
        "out_max", [P, NTILES * 8], mybir.dt.float32, kind="ExternalOutput"
    )
    oc_h = nc.dram_tensor(
        "out_cidx", [P, NTILES * 8], mybir.dt.uint32, kind="ExternalOutput"
    )
    ow_h = nc.dram_tensor(
        "out_widx", [P, NTILES * 8], mybir.dt.uint32, kind="ExternalOutput"
    )
    x = x_h.ap()
    x_flat = x.rearrange("r (c e) -> (r c) e", e=CHUNK)

    with tile.TileContext(nc) as tc:
        with (
            tc.tile_pool(name="xin", bufs=4) as xpool,
            tc.tile_pool(name="seg", bufs=2) as segpool,
            tc.tile_pool(name="gath", bufs=3) as gpool,
            tc.tile_pool(name="ofs", bufs=2) as opool,
            tc.tile_pool(name="persist", bufs=1) as ppool,
        ):
            res_max = ppool.tile([P, NTILES * 8], mybir.dt.float32)
            res_cidx = ppool.tile([P, NTILES * 8], mybir.dt.uint32)
            res_widx = ppool.tile([P, NTILES * 8], mybir.dt.uint32)
            # last tile's widx is host-refined; zero its slice so the final
            # DMA never reads uninitialized SBUF (keeps CoreSim strict checks
            # green; hardware output buffers are pre-zeroed anyway)
            nc.vector.memset(res_widx[:, (NTILES - 1) * 8 :], 0)
            # base_tbl[p, t] = (t*128 + p) * NCHUNK : row index into x_flat
            base_tbl = ppool.tile([P, NTILES], mybir.dt.uint32)
            nc.gpsimd.iota(
                base_tbl[:],
                pattern=[[P * NCHUNK, NTILES]],
                base=0,
                channel_multiplier=NCHUNK,
            )

            # (max8_ap, gath_tile, widx8_ap) of the previous iteration: the
            # within-chunk MaxIndex is software-pipelined one iteration late so
            # the DVE never stalls on the indirect-gather round trip.
            pending = None
            for t in range(NTILES):
                segmax = segpool.tile([P, NCHUNK], mybir.dt.float32)
                if t < NTILES - 2:
                    # one full-tile DMA, alternating the two HWDGE rings
                    # (SP / ACT) per tile: hides per-DMA completion latency
                    # and keeps the two concurrent streams in distant DRAM
                    # regions. (Measured dead ends: column-halving a tile
                    # across both rings thrashes the same DRAM rows ~340GB/s;
                    # partition-halving runs at half SBUF-port width.)
                    xt = xpool.tile([P, C], mybir.dt.float32, tag="xt")
                    dma_eng = nc.sync if t % 2 == 0 else nc.scalar
                    dma_eng.dma_start(xt[:], x[t * P : (t + 1) * P, :])
                    nc.vector.reduce_max(
                        out=segmax[:],
                        in_=xt[:].rearrange("p (c e) -> p c e", e=CHUNK),
                        axis=mybir.AxisListType.X,
                    )
                else:
                    # last two tiles: quarter the loads into SEPARATE tiles
                    # (shared slot tag -> no extra SBUF, per-quarter deps) so
                    # the reduces overlap the remaining stream and the DVE
                    # enters the tail with no backlog
                    QS = 4
                    cq = NCHUNK // QS  # 22 chunks per quarter
                    wq = cq * CHUNK  # 2750 columns
                    for q in range(QS):
                        xq = xpool.tile([P, wq], mybir.dt.float32, tag="xt")
                        dma_eng = nc.sync if q % 2 == 0 else nc.scalar
                        dma_eng.dma_start(
                            xq[:],
                            x[t * P : (t + 1) * P, q * wq : (q + 1) * wq],
                        )
                        nc.vector.reduce_max(
                            out=segmax[:, q * cq : (q + 1) * cq],
                            in_=xq[:].rearrange("p (c e) -> p c e", e=CHUNK),
                            axis=mybir.AxisListType.X,
                        )

                max8 = res_max[:, t * 8 : (t + 1) * 8]
                nc.vector.max(out=max8, in_=segmax[:])
                cidx8 = res_cidx[:, t * 8 : (t + 1) * 8]
                nc.vector.max_index(cidx8, max8, segmax[:])

                if t < NTILES - 1:
                    # last tile's within-chunk argmax is refined on the host
                    # (128 rows/core) so no gather round trip lands in the tail
                    gofs = opool.tile([P, 1], mybir.dt.uint32)
                    nc.vector.tensor_tensor(
                        out=gofs[:],
                        in0=base_tbl[:, t : t + 1],
                        in1=cidx8[:, :1],
                        op=mybir.AluOpType.add,
                    )

                    gath = gpool.tile([P, CHUNK], mybir.dt.float32)
                    nc.gpsimd.indirect_dma_start(
                        out=gath[:],
                        out_offset=None,
                        in_=x_flat,
                        in_offset=bass.IndirectOffsetOnAxis(ap=gofs[:, :1], axis=0),
                    )
                if pending is not None:
                    p_max8, p_gath, p_widx8 = pending
                    nc.vector.max_index(p_widx8, p_max8, p_gath[:])
                if t < NTILES - 1:
                    pending = (max8, gath, res_widx[:, t * 8 : (t + 1) * 8])
                else:
                    pending = None

            # res_widx complete before the stream tail -> this DMA fires early
            nc.sync.dma_start(ow_h.ap()[:, :], res_widx[:])
            nc.sync.dma_start(om_h.ap()[:, :], res_max[:])
            nc.sync.dma_start(oc_h.ap()[:, :], res_cidx[:])
    nc.compile()
    return nc


def get_nc():
    global _NC
    if _NC is None:
        _NC = _build_nc()
    return _NC


def _postprocess(tok, maxp):
    """Mirror of the reference merge/pack given per-step argmax + max prob."""
    max_lp = np.log(maxp + EPS)  # == max_c log(p + eps), f32
    scores = -np.sum(max_lp, axis=1, keepdims=True, dtype=np.float32)

    prev = np.empty_like(tok)
    prev[:, 0] = -1
    prev[:, 1:] = tok[:, :-1]
    keep = (tok != prev) & (tok != BLANK)
    pos = np.cumsum(keep, axis=1, dtype=np.int64) - 1
    idx = np.where(keep, pos, T)  # dropped tokens scatter to dump column T
    out = np.full((B, T + 1), -1, dtype=np.int32)
    out[np.arange(B)[:, None], idx] = tok
    decoded = out[:, :T]
    return decoded, scores


def core_outputs_to_maxtok(out_max, out_cidx, out_widx, shard):
    """[P, NTILES*8] device outputs + the core's input shard ->
    (maxv [ROWS], tok [ROWS]) in row order r = t*P + p."""
    mx = out_max[:, ::8]  # [P, NTILES] slot 0 of each Max8 group
    ci = out_cidx[:, ::8].astype(np.int64)
    wi = out_widx[:, ::8].astype(np.int64).copy()
    # last tile's within-chunk index is not computed on device; refine from
    # the winning chunk (P rows x CHUNK elements) on host
    tl = NTILES - 1
    rows = tl * P + np.arange(P)
    cols = ci[:, tl][:, None] * CHUNK + np.arange(CHUNK)[None, :]
    wi[:, tl] = shard[rows[:, None], cols].argmax(axis=1)
    gi = ci * CHUNK + wi  # [P, NTILES] global argmax
    # row r = t*128 + p  ->  [p, t] -> transpose -> flat r
    return mx.T.reshape(-1), gi.T.reshape(-1)


def kernel(inputs):
    global LAST_RESULTS
    from concourse.bass_utils import run_bass_kernel_spmd

    x = np.ascontiguousarray(np.asarray(inputs, dtype=np.float32)).reshape(B, T, C)
    nc = get_nc()
    shards = x.reshape(N_CORES, ROWS, C)
    in_maps = [{"x": shards[k]} for k in range(N_CORES)]
    res = run_bass_kernel_spmd(nc, in_maps, core_ids=list(range(N_CORES)))
    LAST_RESULTS = res

    maxp = np.empty((N_CORES, B_PER_CORE, T), np.float32)
    tok = np.empty((N_CORES, B_PER_CORE, T), np.int32)
    for k in range(N_CORES):
        rk = res.results[k]
        mv, gi = core_outputs_to_maxtok(
            rk["out_max"], rk["out_cidx"], rk["out_widx"], shards[k]
        )
        maxp[k] = mv.reshape(B_PER_CORE, T)
        tok[k] = gi.reshape(B_PER_CORE, T).astype(np.int32)

    return _postprocess(tok.reshape(B, T), maxp.reshape(B, T))
